# revision 41
# baseline (speedup 1.0000x reference)
"""Trainium2 Bass kernel for nn_SparseMoEBlock (expert-choice MoE, E=8 experts on 8 cores).

Strategy (expert parallelism, per core c = expert e):
  A. transpose local token shard (S/8 tokens) -> xT
  B. gate logits + softmax for shard -> scores [tok, E]; transpose -> [E, tok]
  C. AllToAll: core c receives expert-c scores for ALL tokens (token order)
  D. exact top-K threshold via 7-pass 16-ary search (probes on DVE, counts via PE)
  E. mask + PE-matmul prefix-sum compaction -> (token_id, gating) pair scatter
  F. dma_gather token rows, PE-transpose, tiled expert MLP (gelu-tanh), gating scale
  G. capacity predictor (silu MLP) on local shard
  Host: scatter-add combine of per-expert outputs (disjoint per core), reshape.
"""
import sys
import numpy as np

sys.path.insert(0, "/opt/trn_rl_repo")

import concourse.bass as bass
from concourse import bacc
import concourse.mybir as mybir
from concourse.tile import TileContext
from concourse import library_config

F32 = mybir.dt.float32
F32R = mybir.dt.float32r
I16 = mybir.dt.int16
U32 = mybir.dt.uint32
AF = mybir.ActivationFunctionType
OP = mybir.AluOpType

NCORES = 8


class Cfg:
    def __init__(self, S=8192, D=1024, H=4096, E=8, tokb=512, passes=9,
                 mm_fp32r=False, native_gelu=True):
        assert E == NCORES
        self.S, self.D, self.H, self.E = S, D, H, E
        self.K = (S // E) * 2
        self.Ssh = S // NCORES
        self.F = S // 128          # bisect tile free dim (token = p*F + f)
        self.Fsh = self.Ssh // 128
        self.Dc = D // 128
        self.Hc = H // 128
        self.TOKB = min(tokb, self.K)
        self.NB = self.K // self.TOKB
        self.NS = max(1, self.TOKB // 512)   # 512-token psum splits per block
        self.NSW = min(512, self.TOKB)       # moving width
        self.passes = passes
        self.mm_fp32r = mm_fp32r
        self.native_gelu = native_gelu
        assert self.K % self.TOKB == 0 and self.TOKB % 128 == 0
        assert self.Ssh % self.NSW == 0


def build(cfg: Cfg):
    S, D, H, E, K = cfg.S, cfg.D, cfg.H, cfg.E, cfg.K
    F, Ssh, Fsh, Dc, Hc = cfg.F, cfg.Ssh, cfg.Fsh, cfg.Dc, cfg.Hc
    TOKB, NB, NS, NSW = cfg.TOKB, cfg.NB, cfg.NS, cfg.NSW
    TT = TOKB // 128   # 128-token tiles per block
    Kf = float(K)
    NPR = 7
    MDT = F32R if cfg.mm_fp32r else F32

    nc = bacc.Bacc(num_devices=NCORES)

    # ---------------- inputs ----------------
    x_full = nc.dram_tensor("x_full", [S, D], F32, kind="ExternalInput")
    x_shard = nc.dram_tensor("x_shard", [Ssh, D], F32, kind="ExternalInput")
    gate_wT = nc.dram_tensor("gate_wT", [D, E], F32, kind="ExternalInput")
    cap_w1 = nc.dram_tensor("cap_w1", [D, D], F32, kind="ExternalInput")
    cap_b1 = nc.dram_tensor("cap_b1", [D], F32, kind="ExternalInput")
    cap_w2 = nc.dram_tensor("cap_w2", [D, E], F32, kind="ExternalInput")
    cap_b2 = nc.dram_tensor("cap_b2", [E], F32, kind="ExternalInput")
    w1 = nc.dram_tensor("w1", [D, H], F32, kind="ExternalInput")
    b1 = nc.dram_tensor("b1", [H], F32, kind="ExternalInput")
    w2 = nc.dram_tensor("w2", [H, D], F32, kind="ExternalInput")
    b2 = nc.dram_tensor("b2", [D], F32, kind="ExternalInput")
    ident_in = nc.dram_tensor("ident_in", [128, 128], F32, kind="ExternalInput")
    ltri_in = nc.dram_tensor("ltri_in", [128, 128], F32, kind="ExternalInput")  # strictly lower
    ones_col_in = nc.dram_tensor("ones_col_in", [128, 1], F32, kind="ExternalInput")
    ones_row_in = nc.dram_tensor("ones_row_in", [1, 128], F32, kind="ExternalInput")
    iota15_in = nc.dram_tensor("iota15_in", [1, 15], F32, kind="ExternalInput")
    iota_tok_in = nc.dram_tensor("iota_tok_in", [128, F], F32, kind="ExternalInput")

    # ---------------- outputs ----------------
    outw = nc.dram_tensor("outw", [K, D], F32, kind="ExternalOutput")
    idxgat = nc.dram_tensor("idxgat", [K + 1, 64], F32, kind="ExternalOutput")
    dst_tmp = nc.dram_tensor("dst_tmp", [S], I16)
    ones_sh = nc.dram_tensor("ones_sh", [128, F], F32, kind="ExternalOutput")
    cap_sh = nc.dram_tensor("cap_sh", [E, Ssh], F32, kind="ExternalOutput")

    # ---------------- collective internals ----------------
    cc_in = nc.dram_tensor("cc_in", [E, Ssh], F32)
    cc_out = nc.dram_tensor("cc_out", [E, Ssh], F32)

    with TileContext(nc) as tc:
        with tc.tile_pool(name="const", bufs=1) as cp, \
             tc.tile_pool(name="persist", bufs=1) as pp, \
             tc.tile_pool(name="work", bufs=2) as wp, \
             tc.tile_pool(name="big", bufs=1) as bp, \
             tc.tile_pool(name="wstream", bufs=2) as ws, \
             tc.tile_pool(name="ps1", bufs=2, space="PSUM") as ps1, \
             tc.tile_pool(name="ps2", bufs=2, space="PSUM") as ps2, \
             tc.tile_pool(name="psT", bufs=2, space="PSUM") as psT, \
             tc.tile_pool(name="psS", bufs=2, space="PSUM") as psS:

            # ---- constants
            ident = cp.tile([128, 128], F32, tag="ident")
            nc.sync.dma_start(ident[:], ident_in[:])
            ltri = cp.tile([128, 128], F32, tag="ltri")
            nc.sync.dma_start(ltri[:], ltri_in[:])
            ones_col = cp.tile([128, 1], F32, tag="ones_col")
            nc.sync.dma_start(ones_col[:], ones_col_in[:])
            ones_row = cp.tile([1, 128], F32, tag="ones_row")
            nc.sync.dma_start(ones_row[:], ones_row_in[:])
            iota15 = cp.tile([1, 15], F32, tag="iota15")
            nc.sync.dma_start(iota15[:], iota15_in[:])
            iota_tok = cp.tile([128, F], F32, tag="iota_tok")
            nc.sync.dma_start(iota_tok[:], iota_tok_in[:])
            gw_t = cp.tile([128, Dc, E], F32, tag="gw")
            nc.sync.dma_start(gw_t[:], gate_wT[:].rearrange("(j p) e -> p j e", p=128))
            b1sb = cp.tile([128, Hc], F32, tag="b1")
            nc.sync.dma_start(b1sb[:], b1[:].rearrange("(m p) -> p m", p=128))
            b2sb = cp.tile([128, Dc], F32, tag="b2")
            nc.sync.dma_start(b2sb[:], b2[:].rearrange("(m p) -> p m", p=128))
            cb1sb = cp.tile([128, Dc], F32, tag="cb1")
            nc.sync.dma_start(cb1sb[:], cap_b1[:].rearrange("(m p) -> p m", p=128))
            cb2sb = cp.tile([E, 1], F32, tag="cb2")
            nc.sync.dma_start(cb2sb[:], cap_b2[:].unsqueeze(1))

            # ---- A: transpose local shard -> xT_all [128, Dc, Ssh]
            xT_all = pp.tile([128, Dc, Ssh], F32, tag="xT")
            xsb = None
            for i in range(Fsh):
                if i % TT == 0:
                    xsb = bp.tile([128, TT, D], F32, tag="blk16k")
                xs = xsb[:, i % TT, :]
                nc.sync.dma_start(xs[:], x_shard[i * 128:(i + 1) * 128, :])
                for j0 in range(0, Dc, 4):
                    qn = min(4, Dc - j0)
                    pt = psT.tile([128, 512], F32, tag="pt")
                    for q in range(qn):
                        j = j0 + q
                        nc.tensor.transpose(pt[:, q * 128:(q + 1) * 128],
                                            xs[:, j * 128:(j + 1) * 128], ident[:])
                    for q in range(qn):
                        j = j0 + q
                        nc.vector.tensor_copy(xT_all[:, j, i * 128:(i + 1) * 128],
                                              pt[:, q * 128:(q + 1) * 128])

            # ---- B: gate + softmax per shard tile; build scT [E, Ssh]
            scT = pp.tile([E, Ssh], F32, tag="scT")
            for i in range(Fsh):
                psg = psS.tile([128, 512], F32, tag="small")
                g = psg[:, 0:E]
                for j in range(Dc):
                    nc.tensor.matmul(g, xT_all[:, j, i * 128:(i + 1) * 128],
                                     gw_t[:, j, :], start=(j == 0), stop=(j == Dc - 1))
                rmax = wp.tile([128, 1], F32, tag="rmax")
                nc.vector.tensor_reduce(rmax[:], g, axis=mybir.AxisListType.X, op=OP.max)
                xm = wp.tile([128, E], F32, tag="xm")
                nc.vector.tensor_scalar(xm[:], g, rmax[:], None, op0=OP.subtract)
                ex = wp.tile([128, E], F32, tag="ex")
                nc.scalar.activation(ex[:], xm[:], AF.Exp)
                rsum = wp.tile([128, 1], F32, tag="rsum")
                nc.vector.tensor_reduce(rsum[:], ex[:], axis=mybir.AxisListType.X, op=OP.add)
                rinv = wp.tile([128, 1], F32, tag="rinv")
                nc.vector.reciprocal(rinv[:], rsum[:])
                sci = wp.tile([128, E], F32, tag="sci")
                nc.vector.tensor_scalar(sci[:], ex[:], rinv[:], None, op0=OP.mult)
                pst = psS.tile([128, 512], F32, tag="small")
                nc.tensor.transpose(pst[0:E, 0:128], sci[:], ident[:])
                nc.vector.tensor_copy(scT[:, i * 128:(i + 1) * 128], pst[0:E, 0:128])
            nc.sync.dma_start(cc_in[:], scT[:])
            # ---- C: AllToAll -> expert-c scores for all tokens, token order
            nc.gpsimd.collective_compute(
                "AllToAll", OP.bypass,
                replica_groups=[list(range(NCORES))],
                ins=[cc_in[:]], outs=[cc_out[:]],
            )
            # ---- G: capacity predictor on local shard (MDT matmuls)
            # re-transpose the shard into the recycled "xT" slots as MDT
            # (waits for the gate's last read of xT_all via tag WAR)
            xTr = pp.tile([128, Dc, Ssh], MDT, tag="xT")
            xsb2 = None
            for i in range(Fsh):
                if i % TT == 0:
                    xsb2 = bp.tile([128, TT, D], F32, tag="blk16k")
                xs2 = xsb2[:, i % TT, :]
                nc.sync.dma_start(xs2[:], x_shard[i * 128:(i + 1) * 128, :])
                for j0 in range(0, Dc, 4):
                    qn = min(4, Dc - j0)
                    pt = psT.tile([128, 512], F32, tag="pt")
                    for q in range(qn):
                        j = j0 + q
                        nc.tensor.transpose(pt[:, q * 128:(q + 1) * 128],
                                            xs2[:, j * 128:(j + 1) * 128], ident[:])
                    for q in range(qn):
                        j = j0 + q
                        nc.vector.tensor_copy(xTr[:, j, i * 128:(i + 1) * 128],
                                              pt[:, q * 128:(q + 1) * 128])
            cw2r = cp.tile([128, Dc, E], MDT, tag="cw2r")
            (nc.gpsimd if cfg.mm_fp32r else nc.sync).dma_start(
                cw2r[:], cap_w2[:].rearrange("(j p) e -> p j e", p=128))
            for n in range(Ssh // NSW):
                capH = bp.tile([128, Dc, NSW], MDT, tag="capH")
                for m in range(Dc):
                    cw1p = ws.tile([128, Dc, 128], MDT, tag="w1p", bufs=3)
                    (nc.gpsimd if cfg.mm_fp32r else nc.sync).dma_start(
                        cw1p[:], cap_w1[:, m * 128:(m + 1) * 128].rearrange("(j p) h -> p j h", p=128))
                    pm = ps1.tile([128, NSW], F32, tag="mm1")
                    for j in range(Dc):
                        nc.tensor.matmul(pm[:], cw1p[:, j, :],
                                         xTr[:, j, n * NSW:(n + 1) * NSW],
                                         start=(j == 0), stop=(j == Dc - 1))
                    zz = bp.tile([128, NSW], F32, tag="capz")
                    nc.scalar.activation(zz[:], pm[:], AF.Identity, bias=cb1sb[:, m:m + 1])
                    sg = bp.tile([128, NSW], F32, tag="capsg")
                    nc.scalar.activation(sg[:], pm[:], AF.Sigmoid, bias=cb1sb[:, m:m + 1])
                    nc.vector.tensor_tensor(out=capH[:, m, :], in0=zz[:], in1=sg[:], op=OP.mult)
                pm2 = ps2.tile([128, NSW], F32, tag="mm2")
                c2 = pm2[0:E, :]
                for m in range(Dc):
                    nc.tensor.matmul(c2, cw2r[:, m, :], capH[:, m, :],
                                     start=(m == 0), stop=(m == Dc - 1))
                capo = wp.tile([E, NSW], F32, tag="capo", bufs=1)
                nc.scalar.activation(capo[:], c2, AF.Identity, bias=cb2sb[:, 0:1])
                nc.sync.dma_start(cap_sh[:, n * NSW:(n + 1) * NSW], capo[:])


            sc_e = pp.tile([128, F], F32, tag="sc_e")
            nc.sync.dma_start(sc_e[:], cc_out[:].rearrange("e (a p) -> p (e a)", p=128))

            # ---- D: threshold search (7 passes x 15 probes, 16x narrowing)
            lo = wp.tile([1, 1], F32, tag="lo")
            nc.vector.memset(lo[:], 0.0)
            stp = wp.tile([1, 1], F32, tag="stp")
            nc.vector.memset(stp[:], 1.0 / (NPR + 1.0))
            for p in range(cfg.passes):
                prow = wp.tile([1, NPR], F32, tag="prow")
                nc.vector.tensor_scalar(prow[:], iota15[0:1, 0:NPR], stp[0:1, :], None, op0=OP.mult)
                nc.vector.tensor_scalar(prow[:], prow[:], lo[0:1, :], None, op0=OP.add)
                pb = psS.tile([128, 512], F32, tag="small")
                nc.tensor.matmul(pb[:, 0:NPR], ones_row[:], prow[:], start=True, stop=True)
                prb = wp.tile([128, NPR], F32, tag="prb")
                nc.scalar.copy(prb[:], pb[:, 0:NPR])
                ge = bp.tile([128, NPR * F], F32, tag="ge")
                nc.vector.tensor_tensor(
                    out=ge[:].rearrange("p (i f) -> p i f", i=NPR),
                    in0=sc_e[:].unsqueeze(1).broadcast_to([128, NPR, F]),
                    in1=prb[:].unsqueeze(2).broadcast_to([128, NPR, F]),
                    op=OP.is_ge)
                cnt = wp.tile([128, NPR], F32, tag="cnt")
                nc.vector.tensor_reduce(cnt[:], ge[:].rearrange("p (i f) -> p i f", i=NPR),
                                        axis=mybir.AxisListType.X, op=OP.add)
                pc = psS.tile([128, 512], F32, tag="small")
                nc.tensor.matmul(pc[0:1, 0:NPR], ones_col[:], cnt[:], start=True, stop=True)
                cntr = wp.tile([1, NPR], F32, tag="cntr")
                nc.vector.tensor_copy(cntr[:], pc[0:1, 0:NPR])
                gek = wp.tile([1, NPR], F32, tag="gek")
                nc.vector.tensor_scalar(gek[:], cntr[:], Kf, None, op0=OP.is_ge)
                ngf = wp.tile([1, 1], F32, tag="ngf")
                nc.vector.tensor_reduce(ngf[:], gek[:], axis=mybir.AxisListType.X, op=OP.add)
                dlt = wp.tile([1, 1], F32, tag="dlt")
                nc.vector.tensor_scalar(dlt[:], ngf[:], stp[0:1, :], None, op0=OP.mult)
                lo2 = wp.tile([1, 1], F32, tag="lo")
                nc.vector.tensor_scalar(lo2[:], dlt[:], lo[0:1, :], None, op0=OP.add)
                lo = lo2
                stp2 = wp.tile([1, 1], F32, tag="stp")
                nc.vector.tensor_scalar(stp2[:], stp[:], 1.0 / (NPR + 1.0), None, op0=OP.mult)
                stp = stp2

            # ---- E: mask, ones output, compaction scatter
            ptb = psS.tile([128, 512], F32, tag="small")
            nc.tensor.matmul(ptb[:, 0:1], ones_row[:], lo[:], start=True, stop=True)
            tb = wp.tile([128, 1], F32, tag="tb")
            nc.scalar.copy(tb[:], ptb[:, 0:1])
            mask = pp.tile([128, F], F32, tag="mask")
            nc.vector.tensor_scalar(mask[:], sc_e[:], tb[:], None, op0=OP.is_ge)
            nc.sync.dma_start(ones_sh[:], mask[:])

            pcc = psS.tile([128, 512], F32, tag="small")
            nc.tensor.matmul(pcc[0:F, 0:1], mask[:], ones_col[:], start=True, stop=True)
            colcnt = wp.tile([F, 1], F32, tag="colcnt")
            nc.vector.tensor_copy(colcnt[:], pcc[0:F, 0:1])
            pco = psS.tile([128, 512], F32, tag="small")
            nc.tensor.matmul(pco[0:F, 0:1], ltri[0:F, 0:F], colcnt[:], start=True, stop=True)
            coloff = wp.tile([F, 1], F32, tag="coloff")
            nc.vector.tensor_copy(coloff[:], pco[0:F, 0:1])
            por = psS.tile([128, 512], F32, tag="small")
            nc.tensor.matmul(por[0:1, 0:F], coloff[:], ident[0:F, 0:F], start=True, stop=True)
            offrow = wp.tile([1, F], F32, tag="offrow")
            nc.vector.tensor_copy(offrow[:], por[0:1, 0:F])
            pob = psS.tile([128, 512], F32, tag="small")
            nc.tensor.matmul(pob[:, 0:F], ones_row[:], offrow[:], start=True, stop=True)
            offb = wp.tile([128, F], F32, tag="offb")
            nc.scalar.copy(offb[:], pob[:, 0:F])
            ppr = psS.tile([128, 512], F32, tag="small")
            nc.tensor.matmul(ppr[:, 0:F], ltri[:], mask[:], start=True, stop=True)
            pos = wp.tile([128, F], F32, tag="pos")
            nc.vector.tensor_tensor(out=pos[:], in0=ppr[:, 0:F], in1=offb[:], op=OP.add)
            # dst = mask*(pos-K) + K
            nc.vector.tensor_scalar(pos[:], pos[:], Kf, None, op0=OP.subtract)
            nc.vector.tensor_tensor(out=pos[:], in0=pos[:], in1=mask[:], op=OP.mult)
            nc.vector.tensor_scalar(pos[:], pos[:], Kf, None, op0=OP.add)
            dst16 = wp.tile([128, F], I16, tag="dst16")
            nc.vector.tensor_copy(dst16[:], pos[:])
            nc.sync.dma_start(dst_tmp[:].rearrange("(f p) -> p f", p=128), dst16[:])
            payload = bp.tile([128, F, 64], F32, tag="blk16k")
            nc.vector.memset(payload[:], 0.0)
            nc.vector.tensor_copy(payload[:, :, 0], iota_tok[:])
            nc.vector.tensor_copy(payload[:, :, 1], sc_e[:])
            ztile = wp.tile([1, 64], F32, tag="ztile")
            nc.vector.memset(ztile[:], 0.0)
            nc.sync.dma_start(idxgat[:].unsqueeze(0), ztile[:].unsqueeze(1).broadcast_to([1, K + 1, 64]))
            idxw = wp.tile([16, S // 16], I16, tag="idxw", bufs=1)
            nc.sync.dma_start(idxw[:], dst_tmp[:].rearrange("(s q) -> q s", q=16))
            idxwrep = pp.tile([128, S // 16], I16, tag="idxwrep")
            for r in range(8):
                nc.sync.dma_start(idxwrep[r * 16:(r + 1) * 16, :], idxw[:])
            SCH = min(2048, S)
            for c0 in range(S // SCH):
                nc.gpsimd.dma_scatter_add(
                    out_ap=idxgat[:],
                    in_ap=payload[:, c0 * (SCH // 128):(c0 + 1) * (SCH // 128), :],
                    idxs_ap=idxwrep[:, c0 * (SCH // 16):(c0 + 1) * (SCH // 16)],
                    num_idxs=SCH, num_idxs_reg=SCH, elem_size=64)

            # ---- F: idx readback, gather, MLP
            idxf = wp.tile([16, K // 16], F32, tag="idxf", bufs=1)
            nc.sync.dma_start(
                idxf[:], idxgat[0:K, 0:1].rearrange("(f q) one -> q (f one)", q=16))
            idx16 = wp.tile([16, K // 16], I16, tag="idx16")
            nc.vector.tensor_copy(idx16[:], idxf[:])
            idxrep = pp.tile([128, K // 16], I16, tag="idxrep")
            for r in range(8):
                nc.sync.dma_start(idxrep[r * 16:(r + 1) * 16, :], idx16[:])
            gat = pp.tile([128, K // 128], F32, tag="gat")
            nc.sync.dma_start(
                gat[:], idxgat[0:K, 1:2].rearrange("(t p) one -> p (t one)", p=128))

            for b in range(NB):
                xgT = bp.tile([128, Dc, TOKB], MDT, tag="xgT")
                for t in range(TT):
                    g_t = wp.tile([128, 1, D], F32, tag="g_t", bufs=2)
                    gi = b * TT + t
                    nc.gpsimd.dma_gather(
                        g_t[:], x_full[:],
                        idxrep[:, gi * 8:(gi + 1) * 8],
                        num_idxs=128, num_idxs_reg=128, elem_size=D)
                    for j0 in range(0, Dc, 4):
                        qn = min(4, Dc - j0)
                        pt = psT.tile([128, 512], F32, tag="pt")
                        for q in range(qn):
                            j = j0 + q
                            nc.tensor.transpose(pt[:, q * 128:(q + 1) * 128],
                                                g_t[:, 0, j * 128:(j + 1) * 128], ident[:])
                        for q in range(qn):
                            j = j0 + q
                            nc.vector.tensor_copy(xgT[:, j, t * 128:(t + 1) * 128],
                                                  pt[:, q * 128:(q + 1) * 128])
                hT = pp.tile([128, Hc, TOKB], MDT, tag="hT")
                for m in range(Hc):
                    w1p = ws.tile([128, Dc, 128], MDT, tag="w1p", bufs=3)
                    (nc.gpsimd if cfg.mm_fp32r else nc.sync).dma_start(
                        w1p[:], w1[:, m * 128:(m + 1) * 128].rearrange("(j p) h -> p j h", p=128))
                    for n in range(NS):
                        pm = ps1.tile([128, NSW], F32, tag="mm1")
                        for j in range(Dc):
                            nc.tensor.matmul(pm[:], w1p[:, j, :],
                                             xgT[:, j, n * NSW:(n + 1) * NSW],
                                             start=(j == 0), stop=(j == Dc - 1))
                        if cfg.native_gelu:
                            nc.scalar.activation(hT[:, m, n * NSW:(n + 1) * NSW], pm[:],
                                                 AF.Gelu_apprx_tanh, bias=b1sb[:, m:m + 1])
                        else:
                            # composed tanh-approx gelu (sim-safe):
                            z = wp.tile([128, NSW], F32, tag="gelz")
                            nc.scalar.activation(z[:], pm[:], AF.Identity, bias=b1sb[:, m:m + 1])
                            z3 = wp.tile([128, NSW], F32, tag="gelz3")
                            nc.vector.tensor_tensor(out=z3[:], in0=z[:], in1=z[:], op=OP.mult)
                            nc.vector.tensor_tensor(out=z3[:], in0=z3[:], in1=z[:], op=OP.mult)
                            nc.vector.tensor_scalar(z3[:], z3[:], 0.044715, None, op0=OP.mult)
                            nc.vector.tensor_tensor(out=z3[:], in0=z3[:], in1=z[:], op=OP.add)
                            th = wp.tile([128, NSW], F32, tag="gelth")
                            nc.scalar.activation(th[:], z3[:], AF.Tanh,
                                                 scale=float(np.sqrt(2.0 / np.pi)))
                            nc.vector.tensor_scalar(th[:], th[:], 1.0, None, op0=OP.add)
                            nc.vector.tensor_tensor(out=th[:], in0=th[:], in1=z[:], op=OP.mult)
                            nc.vector.tensor_scalar(hT[:, m, n * NSW:(n + 1) * NSW],
                                                    th[:], 0.5, None, op0=OP.mult)
                outT = bp.tile([128, Dc, TOKB], F32, tag="blk16k")
                Hh = max(1, Hc // 4)
                for d in range(Dc):
                    w2ps = []
                    for half in range(Hc // Hh):
                        w2p = ws.tile([128, Hh, 128], MDT, tag="w2p", bufs=3)
                        (nc.gpsimd if cfg.mm_fp32r else nc.sync).dma_start(
                            w2p[:], w2[half * (Hh * 128):(half + 1) * (Hh * 128),
                                       d * 128:(d + 1) * 128].rearrange("(m p) dd -> p m dd", p=128))
                        w2ps.append(w2p)
                    for n in range(NS):
                        pm2 = ps2.tile([128, NSW], F32, tag="mm2")
                        for m in range(Hc):
                            nc.tensor.matmul(pm2[:], w2ps[m // Hh][:, m % Hh, :],
                                             hT[:, m, n * NSW:(n + 1) * NSW],
                                             start=(m == 0), stop=(m == Hc - 1))
                        nc.scalar.activation(outT[:, d, n * NSW:(n + 1) * NSW], pm2[:],
                                             AF.Identity, bias=b2sb[:, d:d + 1])
                for t in range(TT):
                    orow = wp.tile([128, D], F32, tag="orow")
                    gcol = b * TT + t
                    for j0 in range(0, Dc, 4):
                        qn = min(4, Dc - j0)
                        pt = psT.tile([128, 512], F32, tag="pt")
                        for q in range(qn):
                            d = j0 + q
                            nc.tensor.transpose(pt[:, q * 128:(q + 1) * 128],
                                                outT[:, d, t * 128:(t + 1) * 128], ident[:])
                        for q in range(qn):
                            d = j0 + q
                            nc.vector.tensor_scalar(orow[:, d * 128:(d + 1) * 128],
                                                    pt[:, q * 128:(q + 1) * 128],
                                                    gat[:, gcol:gcol + 1], None, op0=OP.mult)
                    nc.sync.dma_start(outw[(b * TOKB + t * 128):(b * TOKB + (t + 1) * 128), :],
                                      orow[:])

    nc.finalize()
    return nc


# ---------------------------------------------------------------------------
# Host-side wrapper
# ---------------------------------------------------------------------------

import os as _os
_FULL = Cfg(mm_fp32r=_os.environ.get("KMOE_FP32R", "1") == "1")
_CACHE = {}


def _r11(a):
    u = np.ascontiguousarray(a, np.float32).view(np.uint32).astype(np.uint64)
    lsb = (u >> 12) & 1
    u = (u + 0x7FF + lsb) & 0xFFFFF000
    return u.astype(np.uint32).view(np.float32)


def make_in_maps(cfg: Cfg, inputs: dict) -> list:
    S, D, H, E, F = cfg.S, cfg.D, cfg.H, cfg.E, cfg.F
    x = np.ascontiguousarray(np.asarray(inputs["x"], np.float32).reshape(S, D))
    gate_w = np.asarray(inputs["gate_weight"], np.float32)
    consts = dict(
        gate_wT=np.ascontiguousarray(gate_w.T),
        cap_w1=np.ascontiguousarray(np.asarray(inputs["cap_w1"], np.float32)),
        cap_b1=np.asarray(inputs["cap_b1"], np.float32),
        cap_w2=np.ascontiguousarray(np.asarray(inputs["cap_w2"], np.float32)),
        cap_b2=np.asarray(inputs["cap_b2"], np.float32),
        ident_in=np.eye(128, dtype=np.float32),
        ltri_in=np.tril(np.ones((128, 128), np.float32), -1).T.copy(),  # L[k,m]=1 iff k<m
        ones_col_in=np.ones((128, 1), np.float32),
        ones_row_in=np.ones((1, 128), np.float32),
        iota15_in=np.arange(1, 16, dtype=np.float32).reshape(1, 15),
        iota_tok_in=np.ascontiguousarray(np.arange(S, dtype=np.float32).reshape(F, 128).T),
    )
    ew1 = np.asarray(inputs["exp_w1"], np.float32)
    eb1 = np.asarray(inputs["exp_b1"], np.float32)
    ew2 = np.asarray(inputs["exp_w2"], np.float32)
    eb2 = np.asarray(inputs["exp_b2"], np.float32)
    Ssh = cfg.Ssh
    in_maps = []
    for c in range(NCORES):
        in_maps.append(dict(
            x_full=x,
            x_shard=np.ascontiguousarray(x[c * Ssh:(c + 1) * Ssh]),
            w1=np.ascontiguousarray(ew1[c]),
            b1=np.ascontiguousarray(eb1[c]),
            w2=np.ascontiguousarray(ew2[c]),
            b2=np.ascontiguousarray(eb2[c]),
            **consts,
        ))
    return in_maps


def assemble(cfg: Cfg, results: list):
    S, D, E, K = cfg.S, cfg.D, cfg.E, cfg.K
    y = np.zeros((S, D), np.float32)
    ones = np.zeros((S, E), np.float32)
    cap = np.zeros((S, E), np.float32)
    Ssh = cfg.Ssh
    for c in range(NCORES):
        r = results[c]
        idx = r["idxgat"][:K, 0].astype(np.int64)
        y[idx] += r["outw"]
        ones[:, c] = r["ones_sh"].T.reshape(S)
        cap[c * Ssh:(c + 1) * Ssh, :] = r["cap_sh"].T
    return y, ones, cap


TRACE = False
LAST_EXEC_NS = None


def kernel(**inputs) -> tuple:
    global LAST_EXEC_NS
    cfg = _FULL
    if "nc" not in _CACHE:
        _CACHE["nc"] = build(cfg)
    nc = _CACHE["nc"]
    in_maps = make_in_maps(cfg, inputs)
    from concourse.bass_utils import run_bass_kernel_spmd
    res = run_bass_kernel_spmd(nc, in_maps, list(range(NCORES)), trace=TRACE)
    LAST_EXEC_NS = res.exec_time_ns
    _CACHE['last_results'] = res.results
    y, ones, cap = assemble(cfg, res.results)
    B, SL = 4, 2048
    return (y.reshape(B, SL, cfg.D),
            ones.reshape(B, SL, cfg.E),
            cap.reshape(B, SL, cfg.E))
